# revision 52
# baseline (speedup 1.0000x reference)
"""Causal multi-head self-attention on 8 Trainium2 NeuronCores.

Sharding: tensor-parallel over heads. Each of the 8 cores owns 2 heads
(128 of the 1024 qkv dims). Per core (all matmul data in bf16, PSUM and
final partials in fp32):

  - QT/KT = (x @ Wq_c^T)^T etc. in transposed-activation layout
    [128 dims, tokens] (Wq pre-scaled by 1/sqrt(hd) on host).
  - V transposed back to natural [tokens, dims] via PE transpose.
  - scores^T = K Q^T per (batch, 2 heads): the two heads' matmuls have
    K=64 contraction and land on disjoint PE row-groups (tile_position
    auto-derived from base partition 0 / 64) so they run concurrently.
  - softmax without max-subtraction (scores are O(+-10), exp safe).
    Causal mask via per-tile widths + one 128x128 staircase mask on the
    diagonal tiles.
  - attn@V: per head M=64, col-paired on disjoint PE col-groups
    (head0 -> out partitions 0:64, head1 -> 64:128) so the pair also
    runs in one pass.  The softmax denominator is NOT folded into this
    matmul; instead DVE accumulates ptsum (sum of exp over k-tiles) and
    one tiny col-paired M=1 matmul pair per q-chunk reduces it over
    partitions.
  - reciprocal of the denominators via the DRAM bounce trick (spread
    512 values over 128 partitions, recip, broadcast back with a
    stride-0 DRAM access pattern).
  - out_partial = A_c^T @ Wo_c^T written per core; host sums the 8
    partials (row-parallel all-reduce done on host).

The emission is a single fused pipeline: while batch b's attention
k-loops run (ACT-heavy: exp), the projections for batch b+1 and the
deferred output-projection chunks of the previous q-chunk are
interleaved into the PE stream as filler, keeping the PE dense.
"""

import os
import numpy as np
from collections import deque
from contextlib import ExitStack

# defensive: recover cleanly if a previous process left a core wedged
os.environ.setdefault("NEURON_RT_RESET_CORES", "1")

import concourse.bass as bass
import concourse.mybir as mybir
import concourse.tile as tile
from concourse import bacc



F32 = mybir.dt.float32
F32R = mybir.dt.float32r
BF16 = mybir.dt.bfloat16
EXP = mybir.ActivationFunctionType.Exp
MULT = mybir.AluOpType.mult
ADD = mybir.AluOpType.add


class Cfg:
    def __init__(self, B=4, S=2048, D=1024, TCH=512, QCH=512, mm_dt="bf16"):
        self.B, self.S, self.D = B, S, D
        self.T = B * S
        self.KT = D // 128          # contraction tiles for projections
        self.TCH = TCH              # token chunk for projections
        self.QCH = QCH              # query chunk for attention
        self.NQC = S // QCH         # q chunks per batch
        self.NCH = S // TCH         # proj token chunks per batch
        self.HD = 64
        self.mm_dt = mm_dt
        assert S % QCH == 0 and QCH % 128 == 0 and S % TCH == 0


def _mmdt(cfg):
    return {"f32r": F32R, "bf16": BF16, "f32": F32}[cfg.mm_dt]


def build_program(cfg: Cfg):
    nc = bacc.Bacc("TRN2", target_bir_lowering=False, debug=False)
    B, S, D, T, KT = cfg.B, cfg.S, cfg.D, cfg.T, cfg.KT
    TCH, QCH, NQC, NCH = cfg.TCH, cfg.QCH, cfg.NQC, cfg.NCH
    NVT = S // 128                 # 128-token V tiles per batch
    MMDT = _mmdt(cfg)
    OC = 512                       # output-projection column chunk
    NI = QCH // 128

    xT_d = nc.dram_tensor("xT", [128, KT, T], MMDT, kind="ExternalInput")
    wq_d = nc.dram_tensor("wq", [128, KT, 128], MMDT, kind="ExternalInput")
    wk_d = nc.dram_tensor("wk", [128, KT, 128], MMDT, kind="ExternalInput")
    wv_d = nc.dram_tensor("wv", [128, KT, 128], MMDT, kind="ExternalInput")
    wo_d = nc.dram_tensor("wo", [128, D], MMDT, kind="ExternalInput")
    mask_d = nc.dram_tensor("mask", [128, 128], MMDT, kind="ExternalInput")
    ident_d = nc.dram_tensor("ident", [128, 128], MMDT, kind="ExternalInput")
    out_d = nc.dram_tensor("out_p", [T, D], F32, kind="ExternalOutput")
    out_r = out_d.rearrange("(n p) o -> p n o", p=128)   # [128, T//128, D]

    with tile.TileContext(nc) as tc, ExitStack() as ctx:
        persist = ctx.enter_context(tc.tile_pool(name="persist", bufs=1))

        # per-batch activation tiles (separate tiles so cross-batch
        # pipelining never creates false dependencies)
        qt_sb = [persist.tile([128, S], MMDT, tag=f"qt{b}", name=f"qt{b}")
                 for b in range(B)]
        kt_sb = [persist.tile([128, S], MMDT, tag=f"kt{b}", name=f"kt{b}")
                 for b in range(B)]
        a_sb = [persist.tile([128, S], MMDT, tag=f"a{b}", name=f"a{b}")
                for b in range(B)]
        # V natural layout + a ones-column per head: the attn@V matmul then
        # also emits the softmax denominator in its last output row.
        v_sb = [persist.tile([128, NVT, 130], MMDT, tag=f"v{b}", name=f"v{b}")
                for b in range(B)]
        wq_sb = persist.tile([128, KT, 128], MMDT, tag="wq")
        wk_sb = persist.tile([128, KT, 128], MMDT, tag="wk")
        wv_sb = persist.tile([128, KT, 128], MMDT, tag="wv")
        wo_sb = persist.tile([128, D], MMDT, tag="wo")
        mask_sb = persist.tile([128, 128], MMDT, tag="mask")
        ident = persist.tile([128, 128], MMDT, tag="ident")
        ones_sb = persist.tile([128, 1], MMDT, tag="ones")
        nc.vector.memset(ones_sb[:], 1.0)
        for b in range(B):
            nc.vector.tensor_copy(
                v_sb[b][:, :, 64:65],
                ones_sb[:, None, :].to_broadcast((128, NVT, 1)))
            nc.vector.tensor_copy(
                v_sb[b][:, :, 129:130],
                ones_sb[:, None, :].to_broadcast((128, NVT, 1)))

        nc.sync.dma_start(wq_sb[:], wq_d[:])
        nc.sync.dma_start(wk_sb[:], wk_d[:])
        nc.sync.dma_start(wv_sb[:], wv_d[:])
        nc.sync.dma_start(wo_sb[:], wo_d[:])
        nc.sync.dma_start(mask_sb[:], mask_d[:])
        nc.sync.dma_start(ident[:], ident_d[:])

        xp = ctx.enter_context(tc.tile_pool(name="xp", bufs=2))
        vtp = ctx.enter_context(tc.tile_pool(name="vtp", bufs=2))
        ptp = ctx.enter_context(tc.tile_pool(name="ptp", bufs=3))
        rcp = ctx.enter_context(tc.tile_pool(name="rcp", bufs=2))
        op = ctx.enter_context(tc.tile_pool(name="op", bufs=2))
        drp = ctx.enter_context(tc.tile_pool(name="drp", bufs=2, space="DRAM"))
        # PSUM: sc 2 banks x2 + att 1 bank x2 + mix 1 bank x2 = 16KB exact
        scp = ctx.enter_context(tc.tile_pool(name="scp", bufs=2, space="PSUM"))
        attp = ctx.enter_context(tc.tile_pool(name="attp", bufs=2, space="PSUM"))
        mixp = ctx.enter_context(tc.tile_pool(name="mixp", bufs=2, space="PSUM"))

        cur = {}

        # ------- projection emit units: (pe_cost_ns, closure) ----------
        def proj_units(b):
            units = []
            for tci in range(NCH):
                t0 = tci * TCH            # batch-local token offset
                g0 = b * S + tci * TCH    # global token offset

                def dma_u(g0=g0, first=(b == 0 and tci == 0)):
                    x_t = xp.tile([128, KT, TCH], MMDT, tag="x", name="x_t")
                    ns = 4 if first else 2
                    step = KT // ns
                    for si in range(ns):
                        nc.gpsimd.dma_start(
                            x_t[:, si * step:(si + 1) * step, :],
                            xT_d[:, si * step:(si + 1) * step, g0:g0 + TCH])
                    cur['x'] = x_t
                units.append(dma_u)

                for kind in ("q", "k", "v"):
                    def proj_u(b=b, t0=t0, kind=kind):
                        x_t = cur['x']
                        w = {"q": wq_sb, "k": wk_sb, "v": wv_sb}[kind]
                        ps = mixp.tile([128, TCH], F32, tag="mix", name="ps")
                        for kt in range(KT):
                            nc.tensor.matmul(
                                ps[:], w[:, kt, :], x_t[:, kt, :],
                                start=(kt == 0), stop=(kt == KT - 1))
                        if kind == "q":
                            nc.vector.tensor_copy(qt_sb[b][:, t0:t0 + TCH], ps[:])
                        elif kind == "k":
                            nc.vector.tensor_copy(kt_sb[b][:, t0:t0 + TCH], ps[:])
                        else:
                            vt = vtp.tile([128, TCH], MMDT, tag="vt", name="vt")
                            nc.vector.tensor_copy(vt[:], ps[:])
                            cur['vt'] = vt
                    units.append(proj_u)

                def tr_u(b=b, t0=t0):
                    vt = cur['vt']
                    for j in range(TCH // 128):
                        tr = mixp.tile([128, 128], MMDT, tag="mix", name="tr")
                        nc.tensor.transpose(
                            tr[:], vt[:, j * 128:(j + 1) * 128], ident[:])
                        ktg = (t0 + j * 128) // 128
                        nc.vector.tensor_copy(
                            v_sb[b][:, ktg, 0:64], tr[:, 0:64])
                        nc.vector.tensor_copy(
                            v_sb[b][:, ktg, 65:129], tr[:, 64:128])
                units.append(tr_u)
            return units

        # ---------- attention with interleaved fillers -----------------
        filler_q = deque()
        pend_bnd = deque()
        pend_wo = deque()

        def pump(i):
            if pend_bnd and i >= 1:
                pend_bnd.popleft()()
            elif pend_wo and i >= 4 and not pend_bnd:
                pend_wo.popleft()()
            elif filler_q:
                pop_filler()

        filler_q.extend(proj_units(0))
        # batch 0 projections run standalone (nothing to interleave into)
        while filler_q:
            filler_q.popleft()()

        def pop_filler():
            filler_q.popleft()()

        for b in range(B):
            if b + 1 < B:
                u = proj_units(b + 1)
                if b + 1 == B - 1:
                    # last batch: emit only its chunk-0 projections during
                    # batch b; defer chunks 1..3 into the last batch's own
                    # (ascending) k-loops, which otherwise have no filler
                    # and stall on the reciprocal bounce before each wo
                    filler_q.extend(u[:5])
                    reserve = u[5:]
                else:
                    filler_q.extend(u)
            last = (b == B - 1)
            if last:
                filler_q.extend(reserve)
            qcs = range(NQC) if last else range(NQC - 1, -1, -1)
            for qc in qcs:
                q0 = qc * QCH
                n_kt = (q0 + QCH) // 128
                if last:
                    # ensure the last batch's projection chunks 0..qc are
                    # emitted before the k-loop that reads them
                    while len(filler_q) > 5 * (NQC - 1 - qc):
                        pop_filler()
                att0 = attp.tile([65, QCH], F32, tag="att0", name="att0",
                                 bufs=1)
                att1 = attp.tile([65, QCH], F32, tag="att1", name="att1",
                                 bufs=1)

                for kti in range(n_kt):
                    k0 = kti * 128
                    co = max(0, k0 - q0)
                    sc = scp.tile([128, 2 * QCH], F32, tag="sc", name="sc")
                    for h in (0, 1):
                        nc.tensor.matmul(
                            sc[:, h * QCH + co:(h + 1) * QCH],
                            kt_sb[b][h * 64:(h + 1) * 64, k0:k0 + 128],
                            qt_sb[b][h * 64:(h + 1) * 64, q0 + co:q0 + QCH],
                            start=True, stop=True)
                    pt = ptp.tile([128, 2, QCH], MMDT, tag="pt", name="pt")
                    sc3 = sc.rearrange("p (h q) -> p h q", h=2)[:, :, co:QCH]
                    nc.scalar.activation(pt[:, :, co:QCH], sc3, EXP)
                    if k0 >= q0:
                        st = pt[:, :, co:co + 128]
                        nc.vector.tensor_tensor(
                            st, st,
                            mask_sb[:, None, :].to_broadcast((128, 2, 128)),
                            MULT)
                    nc.tensor.matmul(
                        att0[:, co:QCH],
                        v_sb[b][:, kti, 0:65], pt[:, 0, co:QCH],
                        start=(kti == 0), stop=(kti == n_kt - 1))
                    nc.tensor.matmul(
                        att1[:, co:QCH],
                        v_sb[b][:, kti, 65:130], pt[:, 1, co:QCH],
                        start=(kti == 0), stop=(kti == n_kt - 1))
                    pump(kti)

                # evacuate att PSUM immediately: frees the single-buffered
                # att banks before the deferred boundary runs
                au = rcp.tile([65, 2, QCH], F32, tag="au", name="au")
                nc.vector.tensor_copy(au[:, 0, :], att0[:])
                nc.vector.tensor_copy(au[:, 1, :], att1[:])

                def boundary(b=b, q0=q0, au=au):
                    # spread the 2x512 denominators (ones-column rows of au)
                    # over 128 partitions via DRAM (recip on few partitions
                    # is ~8cyc/elem/lane), then broadcast back with a
                    # stride-0 DRAM AP.
                    d_dn = drp.tile([1, 2, QCH], F32, tag="ddn", name="d_dn")
                    nc.gpsimd.dma_start(d_dn[:], au[64:65, :, :])
                    sp = rcp.tile([128, 2, NI], F32, tag="sp", name="sp")
                    nc.gpsimd.dma_start(
                        sp[:], d_dn.rearrange("o h (p i) -> p (o h) i", p=128))
                    rcs = rcp.tile([128, 2, NI], F32, tag="rcs", name="rcs")
                    nc.vector.reciprocal(rcs[:], sp[:])
                    d_rc = drp.tile([2, QCH], F32, tag="drc", name="d_rc")
                    nc.gpsimd.dma_start(
                        d_rc.rearrange("h (p i) -> p h i", p=128), rcs[:])
                    bc0 = rcp.tile([64, QCH], F32, tag="bc0", name="bc0")
                    bc1 = rcp.tile([64, QCH], F32, tag="bc1", name="bc1")
                    nc.gpsimd.dma_start(
                        bc0[:],
                        bass.AP(tensor=d_rc.tensor, offset=d_rc.offset,
                                ap=[[0, 64], [1, QCH]]))
                    nc.gpsimd.dma_start(
                        bc1[:],
                        bass.AP(tensor=d_rc.tensor, offset=d_rc.offset + QCH,
                                ap=[[0, 64], [1, QCH]]))
                    cols = slice(q0, q0 + QCH)
                    nc.vector.tensor_tensor(
                        a_sb[b][0:64, cols], au[0:64, 0, :], bc0[:], MULT)
                    a1 = rcp.tile([64, QCH], MMDT, tag="a1", name="a1")
                    nc.vector.tensor_tensor(
                        a1[:], au[0:64, 1, :], bc1[:], MULT)
                    nc.sync.dma_start(a_sb[b][64:128, cols], a1[:])
                pend_bnd.append(boundary)

                for ti in range(QCH // 128):
                    def wo_u(b=b, q0=q0, ti=ti):
                        tl = q0 + ti * 128    # batch-local token offset
                        tt = (b * S + tl) // 128
                        o_sb = op.tile([128, D], F32, tag="osb", name="o_sb")
                        for oc in range(D // OC):
                            o_ps = mixp.tile([128, OC], F32, tag="mix",
                                             name="o_ps")
                            nc.tensor.matmul(
                                o_ps[:],
                                a_sb[b][:, tl:tl + 128],
                                wo_sb[:, oc * OC:(oc + 1) * OC],
                                start=True, stop=True)
                            nc.vector.tensor_copy(
                                o_sb[:, oc * OC:(oc + 1) * OC], o_ps[:])
                        nc.sync.dma_start(out_r[:, tt, :], o_sb[:])
                    pend_wo.append(wo_u)

            # drain remaining projection fillers before the next batch's
            # k-loops (they must be emitted before their data is read)
            while filler_q:
                if pend_bnd:
                    pend_bnd.popleft()()
                elif pend_wo:
                    pend_wo.popleft()()
                pop_filler()

        while pend_bnd or pend_wo:
            if pend_bnd:
                pend_bnd.popleft()()
            elif pend_wo:
                pend_wo.popleft()()

    nc.compile()
    return nc


def prep_inputs(in_features, weight_q, weight_k, weight_v, weight_o, cfg: Cfg,
                n_cores=8):
    """Host-side shard/layout prep. Returns per-core input dicts."""
    B, S, D, T, KT = cfg.B, cfg.S, cfg.D, cfg.T, cfg.KT
    if cfg.mm_dt == "bf16":
        import ml_dtypes
        mmnp = ml_dtypes.bfloat16
    else:
        mmnp = np.float32
    x = np.asarray(in_features, dtype=np.float32).reshape(T, D)
    # xT[p, kt, t] = x[t, kt*128 + p]
    xT = np.ascontiguousarray(
        x.T.reshape(KT, 128, T).transpose(1, 0, 2)).astype(mmnp)
    mask = np.triu(np.ones((128, 128), dtype=np.float32)).astype(mmnp)
    wq = np.asarray(weight_q, dtype=np.float32) * (1.0 / np.sqrt(cfg.HD))
    wk = np.asarray(weight_k, dtype=np.float32)
    wv = np.asarray(weight_v, dtype=np.float32)
    wo = np.asarray(weight_o, dtype=np.float32)

    def wslice(w, c):
        # [128, KT, 128]: ws[p, kt, m] = w[c*128 + m, kt*128 + p]
        ws = w[c * 128:(c + 1) * 128, :]                  # [128, D]
        return np.ascontiguousarray(
            ws.T.reshape(KT, 128, 128).transpose(1, 0, 2))

    in_maps = []
    for c in range(n_cores):
        in_maps.append({
            "xT": xT,
            "wq": wslice(wq, c).astype(mmnp),
            "wk": wslice(wk, c).astype(mmnp),
            "wv": wslice(wv, c).astype(mmnp),
            "wo": np.ascontiguousarray(
                wo[:, c * 128:(c + 1) * 128].T).astype(mmnp),
            "mask": mask,
            "ident": np.eye(128, dtype=mmnp),
        })
    return in_maps


_CACHE = {}


def _get_program(cfg: Cfg):
    key = (cfg.B, cfg.S, cfg.D, cfg.TCH, cfg.QCH, cfg.mm_dt)
    if key not in _CACHE:
        _CACHE[key] = build_program(cfg)
    return _CACHE[key]


def run(inputs, cfg: Cfg, trace=False, trace_kwargs=None):
    import time
    from concourse.bass_utils import run_bass_kernel_spmd
    nc = _get_program(cfg)
    in_maps = prep_inputs(**inputs, cfg=cfg)
    last = None
    for attempt in range(3):
        try:
            res = run_bass_kernel_spmd(
                nc, in_maps, core_ids=list(range(8)), trace=trace,
                **(trace_kwargs or {}))
            break
        except Exception as e:  # transient NRT device wedges happen
            last = e
            time.sleep(10)
    else:
        raise last
    parts = [r["out_p"] for r in res.results]
    out = np.sum(np.stack(parts, 0).astype(np.float64), axis=0)
    return out.astype(np.float32).reshape(cfg.B, cfg.S, cfg.D), res


def kernel(in_features, weight_q, weight_k, weight_v, weight_o):
    cfg = Cfg()
    out, _ = run(dict(in_features=in_features, weight_q=weight_q,
                      weight_k=weight_k, weight_v=weight_v,
                      weight_o=weight_o), cfg)
    return out


# revision 53
# speedup vs baseline: 1.0325x; 1.0325x over previous
"""Causal multi-head self-attention on 8 Trainium2 NeuronCores.

Sharding: tensor-parallel over heads. Each of the 8 cores owns 2 heads
(128 of the 1024 qkv dims). Per core (all matmul data in bf16, PSUM and
final partials in fp32):

  - QT/KT = (x @ Wq_c^T)^T etc. in transposed-activation layout
    [128 dims, tokens] (Wq pre-scaled by 1/sqrt(hd) on host).
  - V transposed back to natural [tokens, dims] via PE transpose.
  - scores^T = K Q^T per (batch, 2 heads): the two heads' matmuls have
    K=64 contraction and land on disjoint PE row-groups (tile_position
    auto-derived from base partition 0 / 64) so they run concurrently.
  - softmax without max-subtraction (scores are O(+-10), exp safe).
    Causal mask via per-tile widths + one 128x128 staircase mask on the
    diagonal tiles.
  - attn@V: per head M=64, col-paired on disjoint PE col-groups
    (head0 -> out partitions 0:64, head1 -> 64:128) so the pair also
    runs in one pass.  The softmax denominator is NOT folded into this
    matmul; instead DVE accumulates ptsum (sum of exp over k-tiles) and
    one tiny col-paired M=1 matmul pair per q-chunk reduces it over
    partitions.
  - reciprocal of the denominators via the DRAM bounce trick (spread
    512 values over 128 partitions, recip, broadcast back with a
    stride-0 DRAM access pattern).
  - out_partial = A_c^T @ Wo_c^T written per core; host sums the 8
    partials (row-parallel all-reduce done on host).

The emission is a single fused pipeline: while batch b's attention
k-loops run (ACT-heavy: exp), the projections for batch b+1 and the
deferred output-projection chunks of the previous q-chunk are
interleaved into the PE stream as filler, keeping the PE dense.
"""

import os
import numpy as np
from collections import deque
from contextlib import ExitStack

# defensive: recover cleanly if a previous process left a core wedged
os.environ.setdefault("NEURON_RT_RESET_CORES", "1")

import concourse.bass as bass
import concourse.mybir as mybir
import concourse.tile as tile
from concourse import bacc



F32 = mybir.dt.float32
F32R = mybir.dt.float32r
BF16 = mybir.dt.bfloat16
EXP = mybir.ActivationFunctionType.Exp
MULT = mybir.AluOpType.mult
ADD = mybir.AluOpType.add


class Cfg:
    def __init__(self, B=4, S=2048, D=1024, TCH=512, QCH=512, mm_dt="bf16"):
        self.B, self.S, self.D = B, S, D
        self.T = B * S
        self.KT = D // 128          # contraction tiles for projections
        self.TCH = TCH              # token chunk for projections
        self.QCH = QCH              # query chunk for attention
        self.NQC = S // QCH         # q chunks per batch
        self.NCH = S // TCH         # proj token chunks per batch
        self.HD = 64
        self.mm_dt = mm_dt
        assert S % QCH == 0 and QCH % 128 == 0 and S % TCH == 0


def _mmdt(cfg):
    return {"f32r": F32R, "bf16": BF16, "f32": F32}[cfg.mm_dt]


def build_program(cfg: Cfg):
    nc = bacc.Bacc("TRN2", target_bir_lowering=False, debug=False)
    B, S, D, T, KT = cfg.B, cfg.S, cfg.D, cfg.T, cfg.KT
    TCH, QCH, NQC, NCH = cfg.TCH, cfg.QCH, cfg.NQC, cfg.NCH
    NVT = S // 128                 # 128-token V tiles per batch
    MMDT = _mmdt(cfg)
    OC = 512                       # output-projection column chunk
    NI = QCH // 128

    xT_d = nc.dram_tensor("xT", [128, KT, T], MMDT, kind="ExternalInput")
    wq_d = nc.dram_tensor("wq", [128, KT, 128], MMDT, kind="ExternalInput")
    wk_d = nc.dram_tensor("wk", [128, KT, 128], MMDT, kind="ExternalInput")
    wv_d = nc.dram_tensor("wv", [128, KT, 128], MMDT, kind="ExternalInput")
    wo_d = nc.dram_tensor("wo", [128, D], MMDT, kind="ExternalInput")
    mask_d = nc.dram_tensor("mask", [128, 128], MMDT, kind="ExternalInput")
    ident_d = nc.dram_tensor("ident", [128, 128], MMDT, kind="ExternalInput")
    out_d = nc.dram_tensor("out_p", [T, D], F32, kind="ExternalOutput")
    out_r = out_d.rearrange("(n p) o -> p n o", p=128)   # [128, T//128, D]

    with tile.TileContext(nc) as tc, ExitStack() as ctx:
        persist = ctx.enter_context(tc.tile_pool(name="persist", bufs=1))

        # per-batch activation tiles (separate tiles so cross-batch
        # pipelining never creates false dependencies)
        qt_sb = [persist.tile([128, S], MMDT, tag=f"qt{b}", name=f"qt{b}")
                 for b in range(B)]
        kt_sb = [persist.tile([128, S], MMDT, tag=f"kt{b}", name=f"kt{b}")
                 for b in range(B)]
        a_sb = [persist.tile([128, S], MMDT, tag=f"a{b}", name=f"a{b}")
                for b in range(B)]
        # V natural layout + a ones-column per head: the attn@V matmul then
        # also emits the softmax denominator in its last output row.
        v_sb = [persist.tile([128, NVT, 130], MMDT, tag=f"v{b}", name=f"v{b}")
                for b in range(B)]
        wq_sb = persist.tile([128, KT, 128], MMDT, tag="wq")
        wk_sb = persist.tile([128, KT, 128], MMDT, tag="wk")
        wv_sb = persist.tile([128, KT, 128], MMDT, tag="wv")
        wo_sb = persist.tile([128, D], MMDT, tag="wo")
        mask_sb = persist.tile([128, 128], MMDT, tag="mask")
        ident = persist.tile([128, 128], MMDT, tag="ident")
        ones_sb = persist.tile([128, 1], MMDT, tag="ones")
        nc.vector.memset(ones_sb[:], 1.0)
        for b in range(B):
            nc.vector.tensor_copy(
                v_sb[b][:, :, 64:65],
                ones_sb[:, None, :].to_broadcast((128, NVT, 1)))
            nc.vector.tensor_copy(
                v_sb[b][:, :, 129:130],
                ones_sb[:, None, :].to_broadcast((128, NVT, 1)))

        nc.sync.dma_start(wq_sb[:], wq_d[:])
        nc.sync.dma_start(wk_sb[:], wk_d[:])
        nc.sync.dma_start(wv_sb[:], wv_d[:])
        nc.sync.dma_start(wo_sb[:], wo_d[:])
        nc.sync.dma_start(mask_sb[:], mask_d[:])
        nc.sync.dma_start(ident[:], ident_d[:])

        xp = ctx.enter_context(tc.tile_pool(name="xp", bufs=2))
        vtp = ctx.enter_context(tc.tile_pool(name="vtp", bufs=2))
        ptp = ctx.enter_context(tc.tile_pool(name="ptp", bufs=3))
        rcp = ctx.enter_context(tc.tile_pool(name="rcp", bufs=2))
        op = ctx.enter_context(tc.tile_pool(name="op", bufs=2))
        drp = ctx.enter_context(tc.tile_pool(name="drp", bufs=2, space="DRAM"))
        # PSUM: sc 2 banks x2 + att 1 bank x2 + mix 1 bank x2 = 16KB exact
        scp = ctx.enter_context(tc.tile_pool(name="scp", bufs=2, space="PSUM"))
        attp = ctx.enter_context(tc.tile_pool(name="attp", bufs=2, space="PSUM"))
        mixp = ctx.enter_context(tc.tile_pool(name="mixp", bufs=2, space="PSUM"))

        cur = {}

        # ------- projection emit units: (pe_cost_ns, closure) ----------
        def proj_units(b):
            units = []
            for tci in range(NCH):
                t0 = tci * TCH            # batch-local token offset
                g0 = b * S + tci * TCH    # global token offset

                def dma_u(g0=g0, first=(b == 0 and tci == 0)):
                    x_t = xp.tile([128, KT, TCH], MMDT, tag="x", name="x_t")
                    ns = 4 if first else 2
                    step = KT // ns
                    for si in range(ns):
                        nc.gpsimd.dma_start(
                            x_t[:, si * step:(si + 1) * step, :],
                            xT_d[:, si * step:(si + 1) * step, g0:g0 + TCH])
                    cur['x'] = x_t
                units.append(dma_u)

                for kind in ("q", "k", "v"):
                    def proj_u(b=b, t0=t0, kind=kind):
                        x_t = cur['x']
                        w = {"q": wq_sb, "k": wk_sb, "v": wv_sb}[kind]
                        ps = mixp.tile([128, TCH], F32, tag="mix", name="ps")
                        for kt in range(KT):
                            nc.tensor.matmul(
                                ps[:], w[:, kt, :], x_t[:, kt, :],
                                start=(kt == 0), stop=(kt == KT - 1))
                        if kind == "q":
                            nc.vector.tensor_copy(qt_sb[b][:, t0:t0 + TCH], ps[:])
                        elif kind == "k":
                            nc.vector.tensor_copy(kt_sb[b][:, t0:t0 + TCH], ps[:])
                        else:
                            vt = vtp.tile([128, TCH], MMDT, tag="vt", name="vt")
                            nc.vector.tensor_copy(vt[:], ps[:])
                            cur['vt'] = vt
                    units.append(proj_u)

                def tr_u(b=b, t0=t0):
                    vt = cur['vt']
                    for j in range(TCH // 128):
                        tr = mixp.tile([128, 128], MMDT, tag="mix", name="tr")
                        nc.tensor.transpose(
                            tr[:], vt[:, j * 128:(j + 1) * 128], ident[:])
                        ktg = (t0 + j * 128) // 128
                        nc.vector.tensor_copy(
                            v_sb[b][:, ktg, 0:64], tr[:, 0:64])
                        nc.vector.tensor_copy(
                            v_sb[b][:, ktg, 65:129], tr[:, 64:128])
                units.append(tr_u)
            return units

        # ---------- attention with interleaved fillers -----------------
        filler_q = deque()
        pend_bnd = deque()
        pend_wo = deque()

        def pump(i):
            if pend_bnd and i >= 1:
                pend_bnd.popleft()()
            elif pend_wo and i >= 4 and not pend_bnd:
                pend_wo.popleft()()
            elif filler_q:
                pop_filler()

        filler_q.extend(proj_units(0))
        # batch 0 projections run standalone (nothing to interleave into)
        while filler_q:
            filler_q.popleft()()

        def pop_filler():
            filler_q.popleft()()

        for b in range(B):
            if b + 1 < B:
                filler_q.extend(proj_units(b + 1))
            for qc in range(NQC - 1, -1, -1):
                q0 = qc * QCH
                n_kt = (q0 + QCH) // 128
                att0 = attp.tile([65, QCH], F32, tag="att0", name="att0",
                                 bufs=1)
                att1 = attp.tile([65, QCH], F32, tag="att1", name="att1",
                                 bufs=1)

                for kti in range(n_kt):
                    k0 = kti * 128
                    co = max(0, k0 - q0)
                    sc = scp.tile([128, 2 * QCH], F32, tag="sc", name="sc")
                    for h in (0, 1):
                        nc.tensor.matmul(
                            sc[:, h * QCH + co:(h + 1) * QCH],
                            kt_sb[b][h * 64:(h + 1) * 64, k0:k0 + 128],
                            qt_sb[b][h * 64:(h + 1) * 64, q0 + co:q0 + QCH],
                            start=True, stop=True)
                    pt = ptp.tile([128, 2, QCH], MMDT, tag="pt", name="pt")
                    sc3 = sc.rearrange("p (h q) -> p h q", h=2)[:, :, co:QCH]
                    nc.scalar.activation(pt[:, :, co:QCH], sc3, EXP)
                    if k0 >= q0:
                        st = pt[:, :, co:co + 128]
                        nc.vector.tensor_tensor(
                            st, st,
                            mask_sb[:, None, :].to_broadcast((128, 2, 128)),
                            MULT)
                    nc.tensor.matmul(
                        att0[:, co:QCH],
                        v_sb[b][:, kti, 0:65], pt[:, 0, co:QCH],
                        start=(kti == 0), stop=(kti == n_kt - 1))
                    nc.tensor.matmul(
                        att1[:, co:QCH],
                        v_sb[b][:, kti, 65:130], pt[:, 1, co:QCH],
                        start=(kti == 0), stop=(kti == n_kt - 1))
                    pump(kti)

                # evacuate att PSUM immediately: frees the single-buffered
                # att banks before the deferred boundary runs
                au = rcp.tile([65, 2, QCH], F32, tag="au", name="au")
                nc.vector.tensor_copy(au[:, 0, :], att0[:])
                nc.vector.tensor_copy(au[:, 1, :], att1[:])

                def boundary(b=b, q0=q0, au=au):
                    # spread the 2x512 denominators (ones-column rows of au)
                    # over 128 partitions via DRAM (recip on few partitions
                    # is ~8cyc/elem/lane), then broadcast back with a
                    # stride-0 DRAM AP.
                    d_dn = drp.tile([1, 2, QCH], F32, tag="ddn", name="d_dn")
                    nc.gpsimd.dma_start(d_dn[:], au[64:65, :, :])
                    sp = rcp.tile([128, 2, NI], F32, tag="sp", name="sp")
                    nc.gpsimd.dma_start(
                        sp[:], d_dn.rearrange("o h (p i) -> p (o h) i", p=128))
                    rcs = rcp.tile([128, 2, NI], F32, tag="rcs", name="rcs")
                    nc.vector.reciprocal(rcs[:], sp[:])
                    d_rc = drp.tile([2, QCH], F32, tag="drc", name="d_rc")
                    nc.gpsimd.dma_start(
                        d_rc.rearrange("h (p i) -> p h i", p=128), rcs[:])
                    bc0 = rcp.tile([64, QCH], F32, tag="bc0", name="bc0")
                    bc1 = rcp.tile([64, QCH], F32, tag="bc1", name="bc1")
                    nc.gpsimd.dma_start(
                        bc0[:],
                        bass.AP(tensor=d_rc.tensor, offset=d_rc.offset,
                                ap=[[0, 64], [1, QCH]]))
                    nc.gpsimd.dma_start(
                        bc1[:],
                        bass.AP(tensor=d_rc.tensor, offset=d_rc.offset + QCH,
                                ap=[[0, 64], [1, QCH]]))
                    cols = slice(q0, q0 + QCH)
                    nc.vector.tensor_tensor(
                        a_sb[b][0:64, cols], au[0:64, 0, :], bc0[:], MULT)
                    a1 = rcp.tile([64, QCH], MMDT, tag="a1", name="a1")
                    nc.vector.tensor_tensor(
                        a1[:], au[0:64, 1, :], bc1[:], MULT)
                    nc.sync.dma_start(a_sb[b][64:128, cols], a1[:])
                pend_bnd.append(boundary)

                for ti in range(QCH // 128):
                    def wo_u(b=b, q0=q0, ti=ti):
                        tl = q0 + ti * 128    # batch-local token offset
                        tt = (b * S + tl) // 128
                        o_sb = op.tile([128, D], F32, tag="osb", name="o_sb")
                        for oc in range(D // OC):
                            o_ps = mixp.tile([128, OC], F32, tag="mix",
                                             name="o_ps")
                            nc.tensor.matmul(
                                o_ps[:],
                                a_sb[b][:, tl:tl + 128],
                                wo_sb[:, oc * OC:(oc + 1) * OC],
                                start=True, stop=True)
                            nc.vector.tensor_copy(
                                o_sb[:, oc * OC:(oc + 1) * OC], o_ps[:])
                        nc.sync.dma_start(out_r[:, tt, :], o_sb[:])
                    pend_wo.append(wo_u)

            # drain remaining projection fillers before the next batch's
            # k-loops (they must be emitted before their data is read)
            while filler_q:
                if pend_bnd:
                    pend_bnd.popleft()()
                elif pend_wo:
                    pend_wo.popleft()()
                pop_filler()

        while pend_bnd or pend_wo:
            if pend_bnd:
                pend_bnd.popleft()()
            elif pend_wo:
                pend_wo.popleft()()

    nc.compile()
    return nc


def prep_inputs(in_features, weight_q, weight_k, weight_v, weight_o, cfg: Cfg,
                n_cores=8):
    """Host-side shard/layout prep. Returns per-core input dicts."""
    B, S, D, T, KT = cfg.B, cfg.S, cfg.D, cfg.T, cfg.KT
    if cfg.mm_dt == "bf16":
        import ml_dtypes
        mmnp = ml_dtypes.bfloat16
    else:
        mmnp = np.float32
    x = np.asarray(in_features, dtype=np.float32).reshape(T, D)
    # xT[p, kt, t] = x[t, kt*128 + p]
    xT = np.ascontiguousarray(
        x.T.reshape(KT, 128, T).transpose(1, 0, 2)).astype(mmnp)
    mask = np.triu(np.ones((128, 128), dtype=np.float32)).astype(mmnp)
    wq = np.asarray(weight_q, dtype=np.float32) * (1.0 / np.sqrt(cfg.HD))
    wk = np.asarray(weight_k, dtype=np.float32)
    wv = np.asarray(weight_v, dtype=np.float32)
    wo = np.asarray(weight_o, dtype=np.float32)

    def wslice(w, c):
        # [128, KT, 128]: ws[p, kt, m] = w[c*128 + m, kt*128 + p]
        ws = w[c * 128:(c + 1) * 128, :]                  # [128, D]
        return np.ascontiguousarray(
            ws.T.reshape(KT, 128, 128).transpose(1, 0, 2))

    in_maps = []
    for c in range(n_cores):
        in_maps.append({
            "xT": xT,
            "wq": wslice(wq, c).astype(mmnp),
            "wk": wslice(wk, c).astype(mmnp),
            "wv": wslice(wv, c).astype(mmnp),
            "wo": np.ascontiguousarray(
                wo[:, c * 128:(c + 1) * 128].T).astype(mmnp),
            "mask": mask,
            "ident": np.eye(128, dtype=mmnp),
        })
    return in_maps


_CACHE = {}


def _get_program(cfg: Cfg):
    key = (cfg.B, cfg.S, cfg.D, cfg.TCH, cfg.QCH, cfg.mm_dt)
    if key not in _CACHE:
        _CACHE[key] = build_program(cfg)
    return _CACHE[key]


def run(inputs, cfg: Cfg, trace=False, trace_kwargs=None):
    import time
    from concourse.bass_utils import run_bass_kernel_spmd
    nc = _get_program(cfg)
    in_maps = prep_inputs(**inputs, cfg=cfg)
    last = None
    for attempt in range(3):
        try:
            res = run_bass_kernel_spmd(
                nc, in_maps, core_ids=list(range(8)), trace=trace,
                **(trace_kwargs or {}))
            break
        except Exception as e:  # transient NRT device wedges happen
            last = e
            time.sleep(10)
    else:
        raise last
    parts = [r["out_p"] for r in res.results]
    out = np.sum(np.stack(parts, 0).astype(np.float64), axis=0)
    return out.astype(np.float32).reshape(cfg.B, cfg.S, cfg.D), res


def kernel(in_features, weight_q, weight_k, weight_v, weight_o):
    cfg = Cfg()
    out, _ = run(dict(in_features=in_features, weight_q=weight_q,
                      weight_k=weight_k, weight_v=weight_v,
                      weight_o=weight_o), cfg)
    return out
